# revision 30
# baseline (speedup 1.0000x reference)
"""Trainium2 Bass kernel for nn_AttentionConv (7x7 local window attention,
8 heads of dim 16, rel position embeddings, B=4 C=O=128 H=W=64).

Sharding: 8 cores = (batch b in 0..3) x (head-type in {h-heads 0-3, w-heads 4-7}).
W-head cores operate on spatially transposed images so rel_w becomes rel_h
structure; host transposes input/output.

Device algorithm per core (4 heads, 64 channels):
  - projections q (+fused rel-projection P7G) and k on channels-partition
    layout via fp32r matmuls.
  - scores for a (8x8 query tile, head pair) as ONE bf16 matmul [K=80,M=128,
    N=196]: contraction rows = [qA|qB|P7M_A|P7M_B|CM16] against
    [kA|kB|E16G|E16G|E16C]. Rel + row-mask ride as mod-16 staircase indicator
    products; col-mask as mod-16 col indicator product (-1e9).
  - exp on ACT straight out of PSUM (fused exit), batched 2 query tiles/op.
  - attn^T via DMA X-bar transpose (bf16), AV + Z via matmul against
    host-pretransposed V (ones column gives Z as output row 32).
  - unnormalized AV blocks + Z returned to host; host divides and assembles.
"""

import numpy as np
import ml_dtypes

import concourse.bass as bass
import concourse.tile as tile
from concourse import bacc, mybir
from concourse.bass_utils import run_bass_kernel_spmd

F32 = mybir.dt.float32
BF16 = mybir.dt.bfloat16
F32R = mybir.dt.float32r
NEG = -1e9
K = 7


# ----------------------------------------------------------------- host prep
def _host_prep(x, Wq, Wk, Wv, rel_h, rel_w):
    """Build per-core input dicts (8 cores).

    Query layout (tile-major): col = 128*qt + 64*s + qq, qt = 8*tr+tc,
    qq = 8*rr+cc, query image pos = (8*tr+rr, 8*tc+cc).
    Key layout (block-col-major, 72-wide padded): key = cb*560 + rp*8 + c,
    cb in 0..8, padded col cp = 8*cb + c (70,71 are dead), padded row rp.
    Window for tile (tr,tc): rows [8tr, 8tr+14) x col-blocks {tc, tc+1}:
    local key = b01*112 + y*8 + c  (y = rp - 8tr).
    """
    x = np.asarray(x, np.float32)
    Wq = np.asarray(Wq, np.float32)
    Wk = np.asarray(Wk, np.float32)
    Wv = np.asarray(Wv, np.float32)
    rel_h2 = np.asarray(rel_h, np.float32).reshape(64, K)
    rel_w2 = np.asarray(rel_w, np.float32).reshape(64, K)
    NEGF = np.float32(NEG)
    bfc = lambda a: a.astype(ml_dtypes.bfloat16)

    # ---- shared constants
    # khc: rhs const rows of KHW [48, 9 blocks * 112]:
    #   rows 0-15  E14 (pairs P7M_A-hi), rows 16-31 E14 (P7M_B-hi),
    #   rows 32-47 E16C: class (8*cb + c) mod 16
    yy16 = np.arange(16)
    khc = np.zeros((48, 9, 14, 8), np.float32)
    for cb in range(9):
        e14 = (np.arange(14)[None, :, None] == yy16[:, None, None]).astype(np.float32)
        khc[0:16, cb] = np.broadcast_to(e14, (16, 14, 8))
        cpm = (8 * cb + np.arange(8)[None, None, :]) % 16
        khc[32:48, cb] = (cpm == yy16[:, None, None]).astype(np.float32)
    khc[16:32] = khc[0:16]
    khc_bf = bfc(khc.reshape(48, 1008))
    # khc2: rhs const rows 0-31 of KHW2 = [E14, E14]
    khc2_bf = bfc(khc[0:32].reshape(32, 1008))
    # CM16 [16, 8192]: col-mask, class (xl - cc) ... stored per q-col:
    # CM16[x', (qt,s,qq)] = 0 if (x' - 8*(qt%8) - cc) % 16 < 7 else NEG
    qt_ = np.arange(64); qq_ = np.arange(64)
    cc_ = qq_ % 8
    cm = np.zeros((16, 64, 64), np.float32)
    for t in range(64):
        cm[:, t, :] = np.where(((yy16[:, None] - 8 * (t % 8) - cc_[None, :]) % 16) < 7,
                               0.0, NEGF)
    CM16 = np.repeat(cm[:, :, None, :], 2, axis=2).reshape(16, 8192)

    maps = []
    for b in range(4):
        for typ in range(2):
            img = x[b] if typ == 0 else np.ascontiguousarray(x[b].transpose(0, 2, 1))
            xpad = np.zeros((128, 70, 70), np.float32)
            xpad[:, 3:67, 3:67] = img
            ch = slice(0, 64) if typ == 0 else slice(64, 128)
            rel = rel_h2 if typ == 0 else rel_w2
            Wqh, Wkh, Wvh = Wq[ch], Wk[ch], Wv[ch]

            # ---- host P7G -> P7M staircase (hi/lo), tile-major
            relblkW = np.zeros((28, 128), np.float32)
            for g in range(4):
                for i in range(K):
                    relblkW[7 * g + i] = (rel[16 * g:16 * g + 16, i][:, None]
                                          * Wqh[16 * g:16 * g + 16]).sum(0)
            P7G = relblkW @ img.reshape(128, 4096)  # [28, 4096] q-image-major
            # tile-major reorder of q columns
            qcol = (np.arange(4096).reshape(8, 8, 8, 8)  # [tr, rr, tc, cc]
                    .transpose(0, 2, 1, 3).reshape(4096))  # image idx for (qt,qq)
            P7Gt = P7G[:, qcol]  # [28, 4096] tile-major
            rr_of = (np.arange(4096) % 64) // 8
            P16 = np.full((4, 16, 4096), NEGF, np.float32)
            for g in range(4):
                P16[g, :7] = P7Gt[7 * g:7 * g + 7]
            yy = np.arange(16)[:, None]
            P7M = np.empty((4, 16, 4096), np.float32)
            for g in range(4):
                P7M[g] = P16[g][(yy - rr_of[None, :]) % 16, np.arange(4096)[None, :]]
            P7Mh = P7M.astype(ml_dtypes.bfloat16).astype(np.float32)
            P7Ml = np.where(np.abs(P7M) > 1e8, 0.0, P7M - P7Mh)
            # qmc1 [48, 8192] per pair: QM1 rows 64-111 = [P7MAh P7MBh CM16]
            # qmc2 [32, 8192] per pair: QM2 rows 0-31  = [P7MAl P7MBl]
            qmc1 = {}
            qmc2 = {}
            for pair in range(2):
                gA, gB = 2 * pair, 2 * pair + 1
                c1 = np.zeros((48, 64, 2, 64), np.float32)
                c1[0:16, :, 0, :] = P7Mh[gA].reshape(16, 64, 64)
                c1[16:32, :, 1, :] = P7Mh[gB].reshape(16, 64, 64)
                c1 = c1.reshape(48, 8192)
                c1[32:48] = CM16
                c2 = np.zeros((32, 64, 2, 64), np.float32)
                c2[0:16, :, 0, :] = P7Ml[gA].reshape(16, 64, 64)
                c2[16:32, :, 1, :] = P7Ml[gB].reshape(16, 64, 64)
                qmc1[pair] = bfc(c1)
                qmc2[pair] = bfc(c2.reshape(32, 8192))

            # ---- host vT block-col-major with ones col [5040, 64] per pair
            vT = xpad.reshape(128, 4900).T @ Wvh.T  # [4900, 64]
            vT2 = np.zeros((70, 9, 8, 64), np.float32)  # [rp, cb, c, ch]
            vTr = vT.reshape(70, 70, 64)
            for cb in range(9):
                w = min(8, 70 - 8 * cb)
                vT2[:, cb, :w] = vTr[:, 8 * cb:8 * cb + w]
            # band-contiguous: [tr, cb, y(14), c(8), ch]: window (tr, tc) =
            # one contiguous 224-row run at (tr*9 + tc)*112
            vTB = np.zeros((8, 9, 14, 8, 64), np.float32)
            for tr in range(8):
                vTB[tr] = vT2[8 * tr:8 * tr + 14].transpose(1, 0, 2, 3)
            vTB = vTB.reshape(8064, 64)
            vts = []
            for pair in range(2):
                v33 = np.zeros((8096, 64), np.float32)
                v33[:8064, 0:32] = vTB[:, 32 * pair:32 * pair + 32]
                v33[:8064, 32] = 1.0
                vts.append(bfc(v33))

            maps.append({
                "xpad": xpad.reshape(128, 4900),
                "wqt": Wqh.T.copy(),
                "wkt": Wkh.T.copy(),
                "vt0": vts[0],
                "vt1": vts[1],
                "qmc1_0": qmc1[0], "qmc1_1": qmc1[1],
                "qmc2_0": qmc2[0], "qmc2_1": qmc2[1],
                "khc": khc_bf, "khc2": khc2_bf,
            })
    return maps


# ------------------------------------------------------------- device kernel
def _build_kernel(debug=False):
    nc = bacc.Bacc("TRN2", target_bir_lowering=False, debug=False,
                   enable_asserts=False, num_devices=8)
    xpad_d = nc.dram_tensor("xpad", [128, 4900], F32, kind="ExternalInput").ap()
    wqt_d = nc.dram_tensor("wqt", [128, 64], F32, kind="ExternalInput").ap()
    wkt_d = nc.dram_tensor("wkt", [128, 64], F32, kind="ExternalInput").ap()
    vt_d = [nc.dram_tensor("vt0", [8096, 64], BF16, kind="ExternalInput").ap(),
            nc.dram_tensor("vt1", [8096, 64], BF16, kind="ExternalInput").ap()]
    qmc1_d = [nc.dram_tensor(f"qmc1_{p}", [48, 8192], BF16,
                             kind="ExternalInput").ap() for p in range(2)]
    qmc2_d = [nc.dram_tensor(f"qmc2_{p}", [32, 8192], BF16,
                             kind="ExternalInput").ap() for p in range(2)]
    khc_d = nc.dram_tensor("khc", [48, 1008], BF16, kind="ExternalInput").ap()
    khc2_d = nc.dram_tensor("khc2", [32, 1008], BF16, kind="ExternalInput").ap()
    blocks_d = nc.dram_tensor("blocks", [64, 128, 128], F32,
                              kind="ExternalOutput").ap()

    with tile.TileContext(nc) as tc:
        with tc.tile_pool(name="persist", bufs=1) as pp:
            WQTs = pp.tile([128, 64], F32, tag="WQTs")
            WQTR = pp.tile([128, 64], F32R, tag="WQTR")
            WKTs = pp.tile([128, 64], F32, tag="WKTs")
            WKTR = pp.tile([128, 64], F32R, tag="WKTR")
            QSTG = pp.tile([64, 4900], BF16, tag="QSTG")
            QSTGL = pp.tile([64, 4900], BF16, tag="QSTGL")
            KSTG = pp.tile([64, 5040], BF16, tag="KSTG")
            KSTGL = pp.tile([64, 5040], BF16, tag="KSTGL")
            QM1 = [pp.tile([112, 8192], BF16, tag=f"QM1_{p}", name=f"QM1_{p}")
                   for p in range(2)]
            QM2 = [pp.tile([64, 8192], BF16, tag=f"QM2_{p}", name=f"QM2_{p}")
                   for p in range(2)]
            QST2 = pp.tile([64, 4096], BF16, tag="QST2")
            QST2L = pp.tile([64, 4096], BF16, tag="QST2L")
            KHW = [pp.tile([112, 1008], BF16, tag=f"KHW{i}", name=f"KHW{i}")
                   for i in range(4)]
            KHW2 = [pp.tile([64, 1008], BF16, tag=f"KHW2_{i}", name=f"KHW2_{i}")
                    for i in range(4)]
            ES = [pp.tile([128, 1024], BF16, tag=f"ES{p}", name=f"ES{p}")
                  for p in range(2)]

            # ---- loads + constant fills
            nc.sync.dma_start(WQTs[:], wqt_d)
            nc.sync.dma_start(WKTs[:], wkt_d)
            for p in range(2):
                nc.sync.dma_start(QM1[p][64:112, :], qmc1_d[p])
                nc.sync.dma_start(QM2[p][0:32, :], qmc2_d[p])
                nc.gpsimd.memset(QM1[p][0:64, :], 0.0)
                nc.gpsimd.memset(QM2[p][32:64, :], 0.0)
                nc.vector.memset(ES[p][:], 0.0)
            for i in range(4):
                nc.sync.dma_start(KHW[i][64:112, :], khc_d)
                nc.sync.dma_start(KHW2[i][0:32, :], khc2_d)
            # dead cols (cb=8, c=6,7) are read (masked) by KHW refresh:
            # keep them finite
            nc.gpsimd.memset(KSTG[:], 0.0)
            nc.gpsimd.memset(KSTGL[:], 0.0)
            nc.vector.tensor_copy(WQTR[:], WQTs[:])
            nc.vector.tensor_copy(WKTR[:], WKTs[:])

            # ---- phase 1: projections; exits write block-col-major K and
            # padded-grid Q (hi + lo)
            KSv = KSTG[:].rearrange("p (cb r c) -> p r cb c", cb=9, r=70, c=8)
            KSvL = KSTGL[:].rearrange("p (cb r c) -> p r cb c", cb=9, r=70, c=8)
            with tc.tile_pool(name="ppsum", bufs=2, space="PSUM") as prj, \
                 tc.tile_pool(name="xpool", bufs=3) as xpool:
                for c10 in range(10):
                    sl = slice(490 * c10, 490 * (c10 + 1))
                    rsl = slice(7 * c10, 7 * c10 + 7)
                    xpt = xpool.tile([128, 490], F32, tag="xpt")
                    nc.sync.dma_start(xpt[:], xpad_d[:, sl])
                    xpr = xpool.tile([128, 490], F32R, tag="xpr")
                    nc.vector.tensor_copy(xpr[:], xpt[:])
                    ps = prj.tile([64, 490], F32, tag="psq")
                    nc.tensor.matmul(ps[:], WQTR[:], xpr[:],
                                     start=True, stop=True)
                    if c10 % 2:
                        nc.scalar.copy(QSTG[:, sl], ps[:])
                    else:
                        nc.vector.tensor_copy(QSTG[:, sl], ps[:])
                    nc.vector.tensor_sub(QSTGL[:, sl], ps[:], QSTG[:, sl])
                    ps2 = prj.tile([64, 490], F32, tag="psk")
                    nc.tensor.matmul(ps2[:], WKTR[:], xpr[:],
                                     start=True, stop=True)
                    psv = ps2[:].rearrange("p (r cp) -> p r cp", r=7)
                    pmain = psv[:, :, 0:64].rearrange(
                        "p r (cb c) -> p r cb c", cb=8)
                    ptail = psv[:, :, 64:70]
                    if c10 % 2:
                        nc.vector.tensor_copy(KSv[:, rsl, 0:8, :], pmain)
                        nc.scalar.copy(KSv[:, rsl, 8, 0:6], ptail)
                    else:
                        nc.scalar.copy(KSv[:, rsl, 0:8, :], pmain)
                        nc.vector.tensor_copy(KSv[:, rsl, 8, 0:6], ptail)
                    nc.vector.tensor_sub(KSvL[:, rsl, 0:8, :], pmain,
                                         KSv[:, rsl, 0:8, :])
                    nc.vector.tensor_sub(KSvL[:, rsl, 8, 0:6], ptail,
                                         KSv[:, rsl, 8, 0:6])

            # ---- phase 2: tile-major shuffle + q fills
            QSr = QSTG[:].rearrange("p (r c) -> p r c", r=70, c=70)
            QSrL = QSTGL[:].rearrange("p (r c) -> p r c", r=70, c=70)
            QT5 = QST2[:].rearrange("p (tr tc rr cc) -> p tr tc rr cc",
                                    tr=8, tc=8, rr=8, cc=8)
            QT5L = QST2L[:].rearrange("p (tr tc rr cc) -> p tr tc rr cc",
                                      tr=8, tc=8, rr=8, cc=8)
            for tr in range(8):
                srcv = QSr[:, 3 + 8 * tr: 3 + 8 * tr + 8, 3:67].rearrange(
                    "p rr (tc cc) -> p tc rr cc", tc=8)
                srcvL = QSrL[:, 3 + 8 * tr: 3 + 8 * tr + 8, 3:67].rearrange(
                    "p rr (tc cc) -> p tc rr cc", tc=8)
                if tr % 2:
                    nc.scalar.copy(QT5[:, tr], srcv)
                    nc.vector.tensor_copy(QT5L[:, tr], srcvL)
                else:
                    nc.vector.tensor_copy(QT5[:, tr], srcv)
                    nc.scalar.copy(QT5L[:, tr], srcvL)
            QDr = [QM1[p][:].rearrange("p (t s w) -> p t s w", t=64, s=2, w=64)
                   for p in range(2)]
            QLr = [QM2[p][:].rearrange("p (t s w) -> p t s w", t=64, s=2, w=64)
                   for p in range(2)]
            for pair in range(2):
                for sub in range(2):
                    head = 2 * pair + sub
                    nc.sync.dma_start(
                        out=QDr[pair][16 * sub:16 * sub + 16, :, sub, :],
                        in_=QST2[16 * head:16 * head + 16, :])
                    nc.sync.dma_start(
                        out=QDr[pair][32 + 16 * sub:48 + 16 * sub, :, sub, :],
                        in_=QST2L[16 * head:16 * head + 16, :])
                    nc.sync.dma_start(
                        out=QLr[pair][32 + 16 * sub:48 + 16 * sub, :, sub, :],
                        in_=QST2[16 * head:16 * head + 16, :])

            if debug:
                pass

            # ---- phase 3: main loop
            KSvv = KSTG[:].rearrange("p (cb rc) -> p cb rc", cb=9, rc=560)
            KSvvL = KSTGL[:].rearrange("p (cb rc) -> p cb rc", cb=9, rc=560)
            with tc.tile_pool(name="spsum", bufs=2, space="PSUM") as sp, \
                 tc.tile_pool(name="apsum", bufs=2, space="PSUM") as ap_, \
                 tc.tile_pool(name="estp", bufs=6) as estp, \
                 tc.tile_pool(name="vtwp", bufs=6) as vtwp, \
                 tc.tile_pool(name="blkp", bufs=3) as blkp:
                khw = {}
                khw2 = {}
                for grp in range(32):
                    tr0 = (2 * grp) // 8
                    if (2 * grp) % 8 == 0:
                        khw = {}
                        khw2 = {}
                        for pair in range(2):
                            t_ = KHW[2 * pair + tr0 % 2]
                            t2_ = KHW2[2 * pair + tr0 % 2]
                            ksl = slice(64 * tr0, 64 * tr0 + 112)
                            tv = t_[:].rearrange("p (cb w) -> p cb w", cb=9)
                            t2v = t2_[:].rearrange("p (cb w) -> p cb w", cb=9)
                            nc.sync.dma_start(
                                out=tv[0:32, :, :],
                                in_=KSvv[32 * pair:32 * pair + 32, :, ksl])
                            nc.sync.dma_start(
                                out=tv[32:64, :, :],
                                in_=KSvv[32 * pair:32 * pair + 32, :, ksl])
                            nc.sync.dma_start(
                                out=t2v[32:64, :, :],
                                in_=KSvvL[32 * pair:32 * pair + 32, :, ksl])
                            khw[pair] = t_
                            khw2[pair] = t2_
                    s4 = sp.tile([128, 1024], F32, tag="s4")
                    s4r = s4[:].rearrange("p (u c) -> p u c", u=4, c=256)
                    for sub in range(2):
                        qt = 2 * grp + sub
                        tcc = qt % 8
                        for pair in range(2):
                            u = 2 * sub + pair
                            nc.tensor.matmul(
                                s4r[:, u, 0:224],
                                QM1[pair][:, 128 * qt: 128 * qt + 128],
                                khw[pair][:, 112 * tcc: 112 * tcc + 224],
                                start=True, stop=False)
                            nc.tensor.matmul(
                                s4r[:, u, 0:224],
                                QM2[pair][:, 128 * qt: 128 * qt + 128],
                                khw2[pair][:, 112 * tcc: 112 * tcc + 224],
                                start=False, stop=True)
                    es = ES[grp % 2]
                    esr = es[:].rearrange("p (u c) -> p u c", u=4, c=256)
                    nc.scalar.activation(
                        esr[:, :, 0:224],
                        s4r[:, :, 0:224],
                        mybir.ActivationFunctionType.Exp)
                    for sub in range(2):
                        qt = 2 * grp + sub
                        tr, tcc = qt // 8, qt % 8
                        avp = None
                        for pair in range(2):
                            u = 2 * sub + pair
                            if pair == 0:
                                avp = ap_.tile([128, 128], F32, tag="avp")
                            est1 = estp.tile([128, 128], BF16, tag="est1")
                            est2 = estp.tile([128, 128], BF16, tag="est2")
                            nc.sync.dma_start(out=est1[:],
                                              in_=es[:, 256 * u: 256 * u + 128],
                                              transpose=True)
                            nc.sync.dma_start(out=est2[:],
                                              in_=es[:, 256 * u + 128: 256 * (u + 1)],
                                              transpose=True)
                            vtw1 = vtwp.tile([128, 64], BF16, tag="vtw1")
                            vtw2 = vtwp.tile([128, 64], BF16, tag="vtw2")
                            w0 = (9 * tr + tcc) * 112
                            nc.scalar.dma_start(out=vtw1[:],
                                                in_=vt_d[pair][w0:w0 + 128, :])
                            nc.scalar.dma_start(
                                out=vtw2[:],
                                in_=vt_d[pair][w0 + 128:w0 + 256, :])
                            base = 64 * pair
                            nc.tensor.matmul(avp[base:base + 64, :], vtw1[:],
                                             est1[:], start=True, stop=False)
                            nc.tensor.matmul(avp[base:base + 64, :], vtw2[:],
                                             est2[:], start=False, stop=True)
                        blk = blkp.tile([128, 128], F32, tag="blk")
                        if sub:
                            nc.vector.tensor_copy(blk[:], avp[:])
                        else:
                            nc.scalar.copy(blk[:], avp[:])
                        nc.gpsimd.dma_start(out=blocks_d[2 * grp + sub], in_=blk[:])

    nc.compile()
    return nc


_NC_CACHE = {}


def kernel(**inputs):
    maps = _host_prep(**inputs)
    if "nc" not in _NC_CACHE:
        _NC_CACHE["nc"] = _build_kernel()
    nc = _NC_CACHE["nc"]
    res = run_bass_kernel_spmd(nc, maps, core_ids=list(range(8)))
    return assemble(res.results)


def assemble(results):
    out = np.zeros((4, 128, 64, 64), np.float32)
    for core in range(8):
        b, typ = core // 2, core % 2
        blocks = results[core]["blocks"]  # [64, 128, 128]
        o = np.zeros((64, 64, 64), np.float32)
        for qt in range(64):
            tr, tcc = qt // 8, qt % 8
            blk = blocks[qt]
            for pair in range(2):
                base = 64 * pair
                av = blk[base:base + 32]
                Z = blk[base + 32]
                gA, gB = 2 * pair, 2 * pair + 1
                oA = av[0:16, 0:64] / Z[None, 0:64]
                oB = av[16:32, 64:128] / Z[None, 64:128]
                rs, cs = slice(8 * tr, 8 * tr + 8), slice(8 * tcc, 8 * tcc + 8)
                o[16 * gA:16 * gA + 16, rs, cs] = oA.reshape(16, 8, 8)
                o[16 * gB:16 * gB + 16, rs, cs] = oB.reshape(16, 8, 8)
        if typ == 0:
            out[b, 0:64] = o
        else:
            out[b, 64:128] = o.transpose(0, 2, 1)
    return out
